# revision 40
# baseline (speedup 1.0000x reference)
"""Causal cosine-sim attention (qk rmsnorm, scale=8) on 8 trn2 NeuronCores.

Shapes: q,k,v [2,16,2048,64] fp32; out [2,16,2048,64] fp32.
Sharding: 32 (batch, head) pairs -> 4 per core (head-parallel); each core
runs an identical SPMD program on its own 4 heads.

v2 redesign (from the 188us baseline trace):
  - The baseline's ~6-12us ACT/PE holes at every half boundary came from
    DVE-FIFO head-of-line blocking: preprocess ops that wait on the
    scratch->DMA-transpose chain sat ahead of the epilogue in the queue,
    and the epilogue's PE transposes (which don't count as HAM-busy)
    cooled the PE clock to 1.2 GHz at every boundary.
  - Engine split: q/k norm chain on DVE (never DMA-blocked: loads are
    prefetched a full head ahead); everything that CAN wait on a DMA
    (kT scale) or is off-path (V masking) goes on GpSimd, emitted last.
  - q_scale folded into k: sim = sum_d qh_d*(kh_d*qs_d*ks_d), so only kT
    is scaled (host passes qs*ks as one vector).
  - Key-padding mask applied by zeroing V rows (incl. the ones column
    that rides the PV matmul as the softmax denominator) instead of an
    ACT bias, freeing the exp to be bias-less.
  - Diagonal causal mask: in-place DVE multiply on pT (bf16, 2x mode)
    instead of GpSimd into a separate tile; PV streams contiguously.
  - Sum-of-squares via bn_stats (one pass) instead of square+reduce.
  - Epilogue deferred: per half, only the PSUM->SBUF copy (bf16) is
    emitted inline; transposes/reciprocals/divides/out-DMA are queued and
    popped 2-per-jb inside the NEXT half's loop, so real matmuls keep the
    HAM warm and nothing stalls the boundary.  Reciprocals batched 4-wide
    via strided APs; divide as one broadcast TT per 4 tiles.
  - jb loop software-pipelined: S^T(jb+1) is emitted before PV(jb) so PE
    never queues behind the exp of the tile it just produced.
"""

import sys
from collections import deque

import numpy as np

try:
    import concourse.bass as bass
except ImportError:
    sys.path.insert(0, "/opt/trn_rl_repo")
    import concourse.bass as bass

import concourse.mybir as mybir
import concourse.tile as tile
from concourse import bacc
from concourse.bass_utils import run_bass_kernel_spmd
from concourse.masks import make_identity

FP32 = mybir.dt.float32
BF16 = mybir.dt.bfloat16

N_CORES = 8
B, H, S, D = 2, 16, 2048, 64
HPC = (B * H) // N_CORES  # heads per core = 4
P = 128
NT = S // P  # 16 key/query blocks
HALF = S // 2
COSINE_SIM_SCALE = 8.0


def build_nc():
    nc = bacc.Bacc("TRN2", target_bir_lowering=False, debug=False)

    q_d = nc.dram_tensor("q", [HPC, S, D], FP32, kind="ExternalInput")
    k_d = nc.dram_tensor("k", [HPC, S, D], FP32, kind="ExternalInput")
    v_d = nc.dram_tensor("v", [HPC, S, D], FP32, kind="ExternalInput")
    ksc_d = nc.dram_tensor("ksc", [D], FP32, kind="ExternalInput")
    mb_d = nc.dram_tensor("mbias", [HPC, S], FP32, kind="ExternalInput")
    out_d = nc.dram_tensor("out", [HPC, S, D], FP32, kind="ExternalOutput")

    AF = mybir.ActivationFunctionType
    ALU = mybir.AluOpType

    with tile.TileContext(nc) as tc:
        with (
            tc.tile_pool(name="constp", bufs=1) as constp,
            tc.tile_pool(name="dramp", bufs=4, space="DRAM") as dramp,
            # bufs=1 is load-bearing: head h+1's q/k loads (and hence its
            # whole norm chain) become READY only after head h's chain has
            # consumed xqk(h).  The Tile scheduler is greedy work-conserving;
            # any data-ready 2.3us chain op would otherwise be slotted into
            # the cracks of boundary-critical DVE sequences.
            tc.tile_pool(name="stagep", bufs=1) as stagep,
            tc.tile_pool(name="sqp", bufs=2) as sqp,
            tc.tile_pool(name="ssp", bufs=6) as ssp,
            tc.tile_pool(name="qnp", bufs=2) as qnp,
            tc.tile_pool(name="qtp", bufs=3) as qtp,
            tc.tile_pool(name="ktp", bufs=3) as ktp,
            tc.tile_pool(name="vbp", bufs=3) as vbp,
            tc.tile_pool(name="zmp", bufs=3) as zmp,
            tc.tile_pool(name="ptp", bufs=6) as ptp,
            tc.tile_pool(name="otsbp", bufs=3) as otsbp,
            tc.tile_pool(name="osbp", bufs=3) as osbp,
            tc.tile_pool(name="recp", bufs=4) as recp,
            tc.tile_pool(name="stp", bufs=2, space="PSUM") as stp,
            tc.tile_pool(name="otp", bufs=2, space="PSUM") as otp,
        ):
            # ---- constants ----
            tri = constp.tile([P, P], BF16, name="tri")
            nc.gpsimd.memset(tri[:], 1.0)
            # keep where col >= row (P^T layout: row=key j, col=query i)
            nc.gpsimd.affine_select(
                out=tri[:],
                in_=tri[:],
                pattern=[[1, P]],
                channel_multiplier=-1,
                base=0,
                compare_op=ALU.is_ge,
                fill=0.0,
            )
            identf = constp.tile([P, P], FP32, name="identf")
            make_identity(nc, identf[:])
            # combined q_scale*k_scale per-dim vector, duplicated over both
            # partition halves (kT rows are d duplicated twice)
            ksc_sb = constp.tile([P, 1], FP32, name="ksc_sb")
            for half in range(2):
                nc.scalar.dma_start(
                    out=ksc_sb[half * D : (half + 1) * D, 0:1],
                    in_=ksc_d[:].rearrange("(d one) -> d one", one=1),
                )

            pre = {}

            # ---- stage A1a: q/k loads into ONE combined staging tile so the
            # whole norm chain below is one op per step (emitted a half+
            # ahead; head 0 uses the otherwise-idle scalar HWDGE queue for
            # the k side to parallelize its startup-critical DMAs) ----
            def loads_qk(h):
                # contiguous-span layout: partition p holds rows 16p..16p+15
                # (4KB contiguous per partition -> full DMA line rate; the
                # 256B-per-partition gather of the (t p) layout ran at ~half
                # bandwidth).  Row s lives at [s//16, (s%16)*D + d]; the norm
                # chain is per-row either way, and the scratch write below
                # maps rows back by the same rule.
                st = {}
                xqk = stagep.tile([P, 2 * NT * D], FP32, tag="stage", name=f"xqk{h}")
                x4 = xqk.rearrange("p (w r d) -> p w r d", w=2, d=D)
                # head 0's q load rides the otherwise-idle scalar HWDGE ring:
                # at startup ~3MB of loads contend for the SDMA engines, and a
                # separate ring gets its packets drained at ~2x the share.
                qeng = nc.scalar if h == 0 else nc.sync
                qeng.dma_start(
                    out=x4[:, 0], in_=q_d[h].rearrange("(p r) d -> p r d", p=P)
                )
                nc.sync.dma_start(
                    out=x4[:, 1], in_=k_d[h].rearrange("(p r) d -> p r d", p=P)
                )
                st["xqk"] = xqk
                pre[h] = st

            # ---- stage A1b: V loads via GpSimd SWDGE (cast fp32->bf16
            # straight into the (D+1)-strided layout; ones column memset).
            # No tensor ops on GpSimd: its SBUF port is shared with the DVE
            # and concurrent TTs slow BOTH engines ~2x.  The key-padding
            # mask rides the exp as a per-partition additive bias, so V
            # needs no masking. ----
            def loads_v(h):
                st = pre[h]
                vb = vbp.tile([P, NT * (D + 1)], BF16, tag="vb", name=f"vb{h}")
                vb3 = vb.rearrange("p (t c) -> p t c", c=D + 1)
                nc.gpsimd.dma_start(
                    out=vb3[:, :, 0:D],
                    in_=v_d[h].rearrange("(t p) d -> p t d", p=P),
                )
                nc.gpsimd.memset(vb3[:, :, D : D + 1], 1.0)
                mbias = zmp.tile([P, NT], FP32, tag="zm", name=f"mb{h}")
                nc.gpsimd.dma_start(
                    out=mbias[:], in_=mb_d[h].rearrange("(t p) -> p t", p=P)
                )
                st["vb"], st["mbias"] = vb, mbias

            # ---- stage A2: norm chain + transposes (emitted a half ahead).
            # For h>0 the whole stage runs at LOWEST priority: the Tile
            # list-scheduler otherwise slots these 2.3us DVE ops into the
            # boundary-critical DVE sequences (ahead of the oT evacuation
            # copies and diag masks the next half's matmuls wait on), which
            # showed up as 7-13us exp holes at every half/head boundary.
            # There is a full half-head of genuine DVE idle for them; the
            # demotion makes the scheduler use it. ----
            def normchain(h):
                if h == 0:
                    return normchain_body(h)
                with tc.high_priority(-10_000_000):
                    return normchain_body(h)

            def normchain_body(h):
                st = pre[h]
                xqk = st["xqk"]
                # combined q+k norm chain, entirely on DVE, one op per step
                # (the DVE never waits on anything but the prefetched loads)
                sqc = sqp.tile([P, 2 * NT * D], FP32, tag="sq", name=f"sqc{h}")
                nc.vector.tensor_mul(sqc[:], xqk[:], xqk[:])
                ss = ssp.tile([P, 2 * NT], FP32, tag="ss", name=f"ss{h}")
                nc.vector.tensor_reduce(
                    out=ss[:],
                    in_=sqc.rearrange("p (g d) -> p g d", d=D),
                    axis=mybir.AxisListType.X,
                    op=ALU.add,
                )
                # rsqrt(ss): Quake magic + 2 Newton iterations on DVE (keeps
                # ScalarE exp-only so its table set loads exactly once)
                rs = ssp.tile([P, 2 * NT], FP32, tag="ss", name=f"rs{h}")
                rsi = rs.bitcast(mybir.dt.int32)
                nc.vector.tensor_scalar(
                    rsi, ss.bitcast(mybir.dt.int32), 1, None, ALU.arith_shift_right
                )
                nc.vector.tensor_scalar(
                    rsi, rsi, -1.0, float(0x5F3759DF), ALU.mult, ALU.add
                )
                tnw = ssp.tile([P, 2 * NT], FP32, tag="ss", name=f"tnw{h}")
                for _ in range(2):
                    nc.vector.tensor_mul(tnw[:], rs[:], rs[:])
                    nc.vector.tensor_mul(tnw[:], tnw[:], ss[:])
                    nc.vector.tensor_scalar(
                        tnw[:], tnw[:], -0.5, 1.5, ALU.mult, ALU.add
                    )
                    nc.vector.tensor_mul(rs[:], rs[:], tnw[:])

                xn = qnp.tile([P, 2 * NT * D], BF16, tag="qn", name=f"xn{h}")
                rs_b = rs.rearrange("p (g one) -> p g one", one=1).broadcast_to(
                    [P, 2 * NT, D]
                )
                nc.vector.tensor_mul(
                    xn.rearrange("p (g d) -> p g d", d=D),
                    xqk.rearrange("p (g d) -> p g d", d=D),
                    rs_b,
                )
                R = S // P
                for which, off in (("q", 0), ("k", NT)):
                    xnw = xn[:, off * D : (off + NT) * D]
                    scratch = dramp.tile(
                        [S, P], BF16, tag="scratch", name=f"sc_{which}{h}"
                    )
                    # both 64-col halves: the duplicate feeds the K=128
                    # doubled-contraction matmul (full-K keeps the PE activity
                    # monitor counting, so the clock stays at 2.4 GHz)
                    for half in range(2):
                        nc.sync.dma_start(
                            out=scratch.rearrange("(p r) c -> p r c", p=P)[
                                :, :, half * D : (half + 1) * D
                            ],
                            in_=xnw.rearrange("p (r d) -> p r d", d=D),
                        )
                    pool = qtp if which == "q" else ktp
                    xt = pool.tile([P, S], BF16, tag=f"{which}T", name=f"{which}T{h}")
                    nc.sync.dma_start_transpose(out=xt[:], in_=scratch[:])
                    st[f"{which}T"] = xt

            # ---- stage B: the only op that waits on the transpose DMA.
            # DVE (GpSimd TENSOR_SCALAR with a vector scalar measured 29us
            # (!) for this tile -- 40x slower).  Emitted after the previous
            # half's loop so the wait is near-zero by the time the DVE
            # reaches it.
            def scale_kT(h):
                kT = pre[h]["kT"]
                nc.vector.tensor_scalar(kT[:], kT[:], ksc_sb[:, 0:1], None, ALU.mult)

            # ---- attention ----
            def half_loop(h, ih, pend):
                st_h = pre[h]
                qT, kT, vb = st_h["qT"], st_h["kT"], st_h["vb"]
                mbias = st_h["mbias"]
                ilo = ih * HALF
                ce = ilo + HALF
                njb = (ilo + HALF) // P  # 8 or 16
                oTh = otp.tile([D + 1, HALF], FP32, tag="ot", name=f"oT{h}_{ih}")
                live = {}
                # per-bank epilogue state: bank g of oTh (cols 512g..512g+511)
                # receives its last PV accumulation at jb == last_jb(g); the
                # moment that PV is emitted, the bank can be evacuated and
                # post-processed while later jb's still accumulate the OTHER
                # bank (different PSUM bank -> concurrent access is legal).
                oT_sb = otsbp.tile([D + 1, HALF], FP32, tag="otsb", name=f"osb{h}_{ih}")
                tp = otp.tile([P, 1024], FP32, tag="ot", name=f"tp{h}_{ih}")
                tp3 = tp.rearrange("p (ib c) -> p ib c", c=P)
                osb = osbp.tile([P, HALF // 2], FP32, tag="osb", name=f"osb2{h}_{ih}")
                osb3 = osb.rearrange("p (ib d) -> p ib d", d=D)

                def mk_tr(ib):
                    def f():
                        nc.tensor.transpose(
                            tp3[:, ib, 0 : D + 1],
                            oT_sb[:, ib * P : (ib + 1) * P],
                            identf[0 : D + 1, 0 : D + 1],
                        )
                    return f

                def mk_div(g):
                    def f():
                        rec = recp.tile([P, 4], FP32, tag="rec", name=f"rc{h}_{ih}_{g}")
                        rec1 = rec.rearrange("p (f one) -> p f one", one=1)
                        nc.vector.reciprocal(
                            rec1, tp3[:, 4 * g : 4 * g + 4, D : D + 1]
                        )
                        nc.vector.tensor_mul(
                            osb3[:, 4 * g : 4 * g + 4, :],
                            tp3[:, 4 * g : 4 * g + 4, 0:D],
                            rec1.broadcast_to([P, 4, D]),
                        )
                    return f

                def mk_out(g):
                    def f():
                        nc.sync.dma_start(
                            out=out_d[h].rearrange("(t p) d -> p t d", p=P)[
                                :, ih * (HALF // P) + 4 * g :
                                ih * (HALF // P) + 4 * g + 4, :
                            ],
                            in_=osb3[:, 4 * g : 4 * g + 4, :],
                        )
                    return f

                def bank_done(g):
                    nc.vector.tensor_copy(
                        oT_sb[:, 512 * g : 512 * g + 512],
                        oTh[:, 512 * g : 512 * g + 512],
                    )
                    for ib in range(4 * g, 4 * g + 4):
                        pend.append(mk_tr(ib))
                    pend.append(mk_div(g))
                    pend.append(mk_out(g))

                last_jbs = {(ilo + 512 * g + 511) // P: g for g in range(2)}
                for step in range(njb + 1):
                    if step < njb:
                        jb = step
                        cs = max(jb * P, ilo)
                        W = ce - cs
                        stt = stp.tile([P, W], FP32, tag="st", name=f"st{h}_{ih}_{jb}")
                        n0 = cs
                        while n0 < ce:
                            w = min(512, ce - n0)
                            nc.tensor.matmul(
                                stt[:, n0 - cs : n0 - cs + w],
                                kT[:, jb * P : (jb + 1) * P],
                                qT[:, n0 : n0 + w],
                                start=True,
                                stop=True,
                            )
                            n0 += w
                        pT = ptp.tile([P, W], BF16, tag="pT", name=f"pT{h}_{ih}_{jb}")
                        nc.scalar.activation(
                            pT[:],
                            stt[:],
                            AF.Exp,
                            scale=COSINE_SIM_SCALE / 2.0,
                            bias=mbias[:, jb : jb + 1],
                        )
                        if cs == jb * P:  # this tile starts at the diagonal
                            nc.vector.tensor_mul(pT[:, 0:P], pT[:, 0:P], tri[:])
                        live[jb] = (pT, cs)
                    if step >= 1:
                        jb = step - 1
                        pT, cs = live.pop(jb)
                        vslice = vb[:, jb * (D + 1) : (jb + 1) * (D + 1)]
                        n0 = cs
                        while n0 < ce:
                            rel = n0 - ilo
                            w = min(ilo + (rel // 512 + 1) * 512, ce) - n0
                            bank = rel // 512
                            last_jb = (ilo + 512 * bank + 511) // P
                            nc.tensor.matmul(
                                oTh[:, rel : rel + w],
                                vslice,
                                pT[:, n0 - cs : n0 - cs + w],
                                start=(jb == 0),
                                stop=(jb == last_jb),
                                skip_group_check=True,
                            )
                            n0 += w
                        if jb in last_jbs:
                            bank_done(last_jbs[jb])
                    for _ in range(2):
                        if pend:
                            pend.popleft()()

            # ---- pipeline ----
            # per head h: q/k loads for h+1 issued at half0 start (sync),
            # norm chain + scratch round-trip for h+1 at half1 start (DVE +
            # sync), V prep for h+1 at half1 start (GpSimd: its queue is
            # clear of diag masks until jb8, ~9us in), kT scale for h+1
            # after half1's loop (DVE: by then its transpose has finished,
            # so it never head-of-line-blocks the queue).
            loads_qk(0)
            loads_v(0)
            normchain(0)
            scale_kT(0)
            pend = deque()
            for h in range(HPC):
                if h + 1 < HPC:
                    loads_qk(h + 1)
                    loads_v(h + 1)
                half_loop(h, 0, pend)
                if h + 1 < HPC:
                    normchain(h + 1)
                half_loop(h, 1, pend)
                if h + 1 < HPC:
                    scale_kT(h + 1)
                del pre[h]
            while pend:
                pend.popleft()()

    nc.compile()
    return nc


def make_in_maps(q, k, v, q_scale, k_scale, mask):
    qf = q.reshape(B * H, S, D)
    kf = k.reshape(B * H, S, D)
    vf = v.reshape(B * H, S, D)
    ksc = (q_scale * k_scale).astype(np.float32)
    mbias_b = np.where(mask, 0.0, -1e30).astype(np.float32)  # [B, S]

    in_maps = []
    for c in range(N_CORES):
        heads = list(range(c * HPC, (c + 1) * HPC))
        in_maps.append(
            {
                "q": np.ascontiguousarray(qf[heads]),
                "k": np.ascontiguousarray(kf[heads]),
                "v": np.ascontiguousarray(vf[heads]),
                "ksc": ksc,
                "mbias": np.ascontiguousarray(
                    np.stack([mbias_b[bh // H] for bh in heads])
                ),
            }
        )
    return in_maps


_NC_CACHE = None


def kernel(q, k, v, q_scale, k_scale, mask):
    global _NC_CACHE
    q = np.asarray(q, dtype=np.float32)
    k = np.asarray(k, dtype=np.float32)
    v = np.asarray(v, dtype=np.float32)
    q_scale = np.asarray(q_scale, dtype=np.float32)
    k_scale = np.asarray(k_scale, dtype=np.float32)
    mask = np.asarray(mask)

    if _NC_CACHE is None:
        _NC_CACHE = build_nc()
    nc = _NC_CACHE

    in_maps = make_in_maps(q, k, v, q_scale, k_scale, mask)
    res = run_bass_kernel_spmd(nc, in_maps, core_ids=list(range(N_CORES)))
    out = np.stack([r["out"] for r in res.results])  # [8, 4, S, D]
    return out.reshape(B, H, S, D).astype(np.float32)
